# revision 10
# baseline (speedup 1.0000x reference)
"""CompressedKVCache kernel for Trainium2 (8 NeuronCores, head-sharded).

Computes, per (b, h) head:
  quantize k/v rows to int4 (per-row min/max affine), scatter into a
  uint8-packed cache at [start_pos : start_pos+L), then dequantize the
  cache prefix [0 : start_pos+L) back to f32.

Sharding: H=32 heads split across 8 cores (4 heads each); everything is
independent per head, no cross-core communication.

Layout: rows are mapped to SBUF partitions in 16-row blocks (partition
p = row // 16), so every DMA descriptor is a large contiguous run
(8 KiB for f32 row-blocks, 1 KiB for the packed cache, 64 B for
scale/zero vectors) instead of the 512 B / 4 B runs a row-interleaved
layout produces.

The packed cache itself is never returned, so the [start, end) region is
quantize->dequantized entirely on-chip; only the [0, start) prefix is read
from the cache inputs.

Work is spread over three engines: DVE (min/max reduces, stats, nibble
unpack, part of the dequant), ACT (quantize with RNE via i32-convert,
part of the dequant), GPSIMD (mult+add dequant chunks).
"""

import sys

sys.path.insert(0, "/opt/trn_rl_repo")

import numpy as np
from concourse import bass, mybir
from concourse import tile
from concourse.bass_utils import run_bass_kernel_spmd

F32 = mybir.dt.float32
I32 = mybir.dt.int32
U32 = mybir.dt.uint32
U8 = mybir.dt.uint8
Alu = mybir.AluOpType
Act = mybir.ActivationFunctionType
AX = mybir.AxisListType
INV15 = float(np.float32(1.0 / 15.0))

B, H, L, D = 2, 32, 2048, 128
MAX_SEQ = 8192
N_CORES = 8
HC = H // N_CORES  # heads per core
RB = 16            # rows per partition block


def _split_multiwait(nc):
    """This container's walrus accepts only ONE sync-wait per instruction;
    Tile's tail drain (and occasionally other insts) carry several. Split
    extras into single-wait EventSemaphore insts inserted just before."""
    for fn in nc.m.functions:
        for blk in fn.blocks:
            out = []
            for ins in blk.instructions:
                si = ins.sync_info
                if si is not None and si.on_wait is not None and len(si.on_wait) > 1:
                    waits = list(si.on_wait)
                    for j, w in enumerate(waits[:-1]):
                        out.append(mybir.InstEventSemaphore(
                            name=f"{ins.name}_sw{j}", ins=[], outs=[],
                            engine=ins.engine,
                            sync_info=mybir.SyncInfo(on_wait=[w], on_update=[])))
                    si.on_wait = [waits[-1]]
                    ins.sync_info = si
                out.append(ins)
            blk.instructions = out


# Engine assignment tables per (pair, tensor) unit index u = pair*2 + kv
# (16 units). "V" = vector/DVE, "A" = scalar/ACT, "G" = gpsimd.
# V's fixed work (reduce/stats/unpack) ends early, so V takes the LATE
# units' dequant blocks to flatten the pipeline drain; A/G take early ones.
DEQF_ENG = ["V"] * 6 + ["A"] * 4 + ["G"] * 3 + ["A"] * 1 + ["V"] * 2
DEQP_ENG = ["G"] * 14 + ["V"] * 2
QUANT_ENG = ["A"] * 14 + ["V"] * 2


def _build(start_pos: int):
    """Trace the per-core Bass kernel for a given start_pos (block layout).

    Per core: xk/xv (B,HC,L,D) f32, prefix packed caches (B,HC,S,64) u8 and
    prefix scale/zero rows (B,HC,S) f32 -> ok/ov (B,HC,S+L,D) f32.
    """
    S = start_pos
    E = S + L
    assert S == 2048 and L == 2048 and E <= MAX_SEQ

    nc = bass.Bass(trn_type="TRN2")

    ins_q, ins_p, ins_sc, ins_zp, outs = {}, {}, {}, {}, {}
    for t in ("k", "v"):
        ins_q[t] = nc.dram_tensor(f"x{t}", [B, HC, L, D], F32, kind="ExternalInput")
        ins_p[t] = nc.dram_tensor(f"p{t}", [B, HC, S, D // 2], U8, kind="ExternalInput")
        ins_sc[t] = nc.dram_tensor(f"sc{t}", [B, HC, S], F32, kind="ExternalInput")
        ins_zp[t] = nc.dram_tensor(f"zp{t}", [B, HC, S], F32, kind="ExternalInput")
        outs[t] = nc.dram_tensor(f"o{t}", [B, HC, E, D], F32, kind="ExternalOutput")

    def chunk_op(eng, out, in0, s1, s2):
        """out = in0*s1 + s2 on the given engine (dequant chunk)."""
        if eng == "V":
            nc.vector.tensor_scalar(out=out, in0=in0, scalar1=s1, scalar2=s2,
                                    op0=Alu.mult, op1=Alu.add)
        elif eng == "G":
            nc.gpsimd.tensor_scalar(out=out, in0=in0, scalar1=s1, scalar2=s2,
                                    op0=Alu.mult, op1=Alu.add)
        else:
            nc.scalar.activation(out=out, in_=in0, func=Act.Identity,
                                 bias=s2, scale=s1)

    with tile.TileContext(nc) as tc:
        with tc.tile_pool(name="xin", bufs=3) as xin, \
             tc.tile_pool(name="qp", bufs=2) as qp, \
             tc.tile_pool(name="op", bufs=2) as op_pool, \
             tc.tile_pool(name="small", bufs=3) as small:
            NP = B * HC
            pairs = [(b, hh) for b in range(B) for hh in range(HC)]
            st = [dict() for _ in range(NP)]  # per-pair tile state

            def emit_dma_in(i):
                b, hh = pairs[i]
                p = st[i]
                for kv, t in enumerate(("k", "v")):
                    x_dram = ins_q[t][b, hh, :, :].rearrange("(p r) d -> p r d", p=128)
                    x = xin.tile([128, RB, D], F32, tag=f"x{kv}", name=f"x{kv}")
                    nc.sync.dma_start(out=x[:, :, :], in_=x_dram)
                    p[f"x{kv}"] = x
                    pk_dram = ins_p[t][b, hh, :, :].rearrange("(p r) d -> p r d", p=128)
                    pk = xin.tile([128, RB, D // 2], U8, tag=f"pk{kv}", name=f"pk{kv}")
                    nc.sync.dma_start(out=pk[:, :, :], in_=pk_dram)
                    p[f"pk{kv}"] = pk
                sc = small.tile([128, 2 * RB], F32, tag="sc", name="sc")
                zp = small.tile([128, 2 * RB], F32, tag="zp", name="zp")
                for kv, t in enumerate(("k", "v")):
                    c0 = kv * RB
                    nc.sync.dma_start(out=sc[:, c0:c0 + RB],
                                      in_=ins_sc[t][b, hh, :].rearrange("(p r) -> p r", p=128))
                    nc.sync.dma_start(out=zp[:, c0:c0 + RB],
                                      in_=ins_zp[t][b, hh, :].rearrange("(p r) -> p r", p=128))
                p["sc"], p["zp"] = sc, zp

            def emit_stage1(i):
                """V-engine work for pair i: reduces, stats, unpack, pnzs."""
                p = st[i]
                mn = small.tile([128, 2 * RB], F32, tag="mn", name="mn")
                mx = small.tile([128, 2 * RB], F32, tag="mx", name="mx")
                for kv in range(2):
                    x = p[f"x{kv}"]
                    c0 = kv * RB
                    nc.vector.tensor_reduce(out=mx[:, c0:c0 + RB], in_=x[:, :, :],
                                            axis=AX.X, op=Alu.max)
                    nc.vector.tensor_reduce(out=mn[:, c0:c0 + RB], in_=x[:, :, :],
                                            axis=AX.X, op=Alu.min)
                scale = small.tile([128, 2 * RB], F32, tag="scale", name="scale")
                nc.vector.tensor_tensor(out=scale[:, :], in0=mx[:, :], in1=mn[:, :],
                                        op=Alu.subtract)
                nc.vector.tensor_scalar(out=scale[:, :], in0=scale[:, :],
                                        scalar1=INV15, scalar2=1e-8,
                                        op0=Alu.mult, op1=Alu.max)
                rcp = small.tile([128, 2 * RB], F32, tag="rcp", name="rcp")
                nc.vector.reciprocal(out=rcp[:, :], in_=scale[:, :])
                zero = small.tile([128, 2 * RB], F32, tag="zero", name="zero")
                nc.vector.scalar_tensor_tensor(out=zero[:, :], in0=mn[:, :],
                                               scalar=-1.0, in1=rcp[:, :],
                                               op0=Alu.mult, op1=Alu.mult)
                nzs = small.tile([128, 2 * RB], F32, tag="nzs", name="nzs")
                nc.vector.scalar_tensor_tensor(out=nzs[:, :], in0=zero[:, :],
                                               scalar=-1.0, in1=scale[:, :],
                                               op0=Alu.mult, op1=Alu.mult)
                pnzs = small.tile([128, 2 * RB], F32, tag="pnzs", name="pnzs")
                nc.vector.scalar_tensor_tensor(out=pnzs[:, :], in0=p["zp"][:, :],
                                               scalar=-1.0, in1=p["sc"][:, :],
                                               op0=Alu.mult, op1=Alu.mult)
                p.update(scale=scale, rcp=rcp, zero=zero, nzs=nzs, pnzs=pnzs)
                for kv in range(2):
                    pk = p[f"pk{kv}"]
                    lohi = xin.tile([128, RB, D], U8, tag=f"lohi{kv}", name=f"lohi{kv}")
                    pk32 = pk[:, :, :].bitcast(U32)
                    nc.vector.tensor_scalar(out=lohi[:, :, 0:D // 2].bitcast(U32),
                                            in0=pk32, scalar1=0x0F0F0F0F,
                                            scalar2=None, op0=Alu.bitwise_and)
                    nc.vector.tensor_scalar(out=lohi[:, :, D // 2:D].bitcast(U32),
                                            in0=pk32, scalar1=4, scalar2=0x0F0F0F0F,
                                            op0=Alu.logical_shift_right,
                                            op1=Alu.bitwise_and)
                    p[f"lohi{kv}"] = lohi

            def emit_quant(i):
                """A-engine (or V) quantize for pair i."""
                p = st[i]
                for kv in range(2):
                    u = i * 2 + kv
                    x = p[f"x{kv}"]
                    c0 = kv * RB
                    q = qp.tile([128, RB, D], I32, tag=f"q{kv}", name=f"q{kv}")
                    for r in range(RB):
                        col = c0 + r
                        if QUANT_ENG[u] == "A":
                            nc.scalar.activation(out=q[:, r, :], in_=x[:, r, :],
                                                 func=Act.Identity,
                                                 bias=p["zero"][:, col:col + 1],
                                                 scale=p["rcp"][:, col:col + 1])
                        else:
                            nc.vector.tensor_scalar(out=q[:, r, :], in0=x[:, r, :],
                                                    scalar1=p["rcp"][:, col:col + 1],
                                                    scalar2=p["zero"][:, col:col + 1],
                                                    op0=Alu.mult, op1=Alu.add)
                    p[f"q{kv}"] = q

            def emit_stage2(i):
                """Dequants + output DMAs for pair i (quant already emitted)."""
                b, hh = pairs[i]
                p = st[i]
                for kv, t in enumerate(("k", "v")):
                    u = i * 2 + kv
                    c0 = kv * RB
                    o = op_pool.tile([128, 2 * RB, D], F32, tag=f"o{kv}", name=f"o{kv}")
                    q = p[f"q{kv}"]
                    lohi = p[f"lohi{kv}"]
                    # prefix dequant first (GPS usually) so its half ships early
                    for r in range(RB):
                        col = c0 + r
                        src = lohi[:, r, :].rearrange("p (two d) -> p two d", two=2)
                        dst = o[:, r, :].rearrange("p (d two) -> p two d", two=2)
                        chunk_op(DEQP_ENG[u], dst, src,
                                 p["sc"][:, col:col + 1], p["pnzs"][:, col:col + 1])
                    opre_dram = outs[t][b, hh, 0:S, :].rearrange("(p r) d -> p r d", p=128)
                    nc.sync.dma_start(out=opre_dram, in_=o[:, 0:RB, :])
                    for r in range(RB):
                        col = c0 + r
                        chunk_op(DEQF_ENG[u], o[:, RB + r, :], q[:, r, :],
                                 p["scale"][:, col:col + 1], p["nzs"][:, col:col + 1])
                    ofr_dram = outs[t][b, hh, S:E, :].rearrange("(p r) d -> p r d", p=128)
                    nc.sync.dma_start(out=ofr_dram, in_=o[:, RB:2 * RB, :])

            emit_dma_in(0)
            emit_dma_in(1)
            for i in range(NP):
                emit_stage1(i)
                emit_quant(i)
                if i + 2 < NP:
                    emit_dma_in(i + 2)
                if i >= 1:
                    emit_stage2(i - 1)
            emit_stage2(NP - 1)

    _split_multiwait(nc)
    return nc


_CACHE = {}


def _get_nc(start_pos: int):
    if start_pos not in _CACHE:
        _CACHE[start_pos] = _build(start_pos)
    return _CACHE[start_pos]


def _install_ntff_hook_shim():
    """The agent image's antenv lacks axon_hooks; recreate it so
    run_bass_kernel_spmd(trace=True) can drive NTFF profiling."""
    import types
    if "antenv.axon_hooks" in sys.modules:
        return
    mod = types.ModuleType("antenv.axon_hooks")
    state = {"hook": None}
    try:
        from trn_agent_boot.trn_boot import _ntff_profile_via_ctypes
        state["hook"] = _ntff_profile_via_ctypes("/opt/axon/libaxon_pjrt.so")
    except Exception:
        pass
    mod.get_axon_ntff_profile_hook = lambda: state["hook"]
    mod.set_axon_ntff_profile_hook = lambda h: state.__setitem__("hook", h)
    sys.modules["antenv.axon_hooks"] = mod


def _kernel_np(k, v, k_cache, v_cache, k_scale, k_zero, v_scale, v_zero, start_pos):
    """Pure-numpy fallback for shapes the bass path doesn't handle."""
    def qp(x):
        mn = x.min(-1, keepdims=True)
        mx = x.max(-1, keepdims=True)
        scale = np.maximum((mx - mn) / np.float32(15.0), np.float32(1e-8))
        zero = -mn / scale
        q = np.clip(np.round(x / scale + zero), 0, 15).astype(np.uint8)
        return (q[..., 0::2] | (q[..., 1::2] << 4)), scale[..., 0], zero[..., 0]

    def dq(p, s, z):
        lo = (p & 15).astype(np.float32)
        hi = ((p >> 4) & 15).astype(np.float32)
        q = np.stack([lo, hi], -1).reshape(p.shape[:-1] + (p.shape[-1] * 2,))
        return (q - z[..., None]) * s[..., None]

    S = int(start_pos)
    E = S + k.shape[2]
    outs = []
    for x, cache, sc, zp in ((k, k_cache, k_scale, k_zero), (v, v_cache, v_scale, v_zero)):
        pp, ps, pz = qp(x)
        cache = cache.copy(); sc = sc.copy(); zp = zp.copy()
        cache[:, :, S:E] = pp
        sc[:, :, S:E] = ps
        zp[:, :, S:E] = pz
        outs.append(dq(cache[:, :, :E], sc[:, :, :E], zp[:, :, :E]))
    return tuple(outs)


def kernel(k, v, k_cache, v_cache, k_scale, k_zero, v_scale, v_zero, start_pos,
           _trace=False):
    k = np.asarray(k, np.float32)
    v = np.asarray(v, np.float32)
    k_cache = np.asarray(k_cache, np.uint8)
    v_cache = np.asarray(v_cache, np.uint8)
    k_scale = np.asarray(k_scale, np.float32)
    k_zero = np.asarray(k_zero, np.float32)
    v_scale = np.asarray(v_scale, np.float32)
    v_zero = np.asarray(v_zero, np.float32)
    S = int(start_pos)

    if k.shape != (B, H, L, D) or S != 2048 or S + L > MAX_SEQ:
        return _kernel_np(k, v, k_cache, v_cache, k_scale, k_zero, v_scale, v_zero, S)

    nc = _get_nc(S)
    E = S + L

    in_maps = []
    for m in range(N_CORES):
        hs = slice(m * HC, (m + 1) * HC)
        im = {
            "xk": np.ascontiguousarray(k[:, hs]),
            "xv": np.ascontiguousarray(v[:, hs]),
            "pk": np.ascontiguousarray(k_cache[:, hs, :S, :]),
            "pv": np.ascontiguousarray(v_cache[:, hs, :S, :]),
            "sck": np.ascontiguousarray(k_scale[:, hs, :S]),
            "zpk": np.ascontiguousarray(k_zero[:, hs, :S]),
            "scv": np.ascontiguousarray(v_scale[:, hs, :S]),
            "zpv": np.ascontiguousarray(v_zero[:, hs, :S]),
        }
        in_maps.append(im)

    if _trace:
        _install_ntff_hook_shim()
    res = run_bass_kernel_spmd(nc, in_maps, list(range(N_CORES)), trace=_trace)

    k_dec = np.empty((B, H, E, D), np.float32)
    v_dec = np.empty((B, H, E, D), np.float32)
    for m in range(N_CORES):
        hs = slice(m * HC, (m + 1) * HC)
        k_dec[:, hs] = res.results[m]["ok"]
        v_dec[:, hs] = res.results[m]["ov"]
    if _trace:
        return (k_dec, v_dec), res
    return k_dec, v_dec


# revision 11
# speedup vs baseline: 1.1309x; 1.1309x over previous
"""CompressedKVCache kernel for Trainium2 (8 NeuronCores, head-sharded).

Computes, per (b, h) head:
  quantize k/v rows to int4 (per-row min/max affine), scatter into a
  uint8-packed cache at [start_pos : start_pos+L), then dequantize the
  cache prefix [0 : start_pos+L) back to f32.

Sharding: H=32 heads split across 8 cores (4 heads each); everything is
independent per head, no cross-core communication.

Layout: rows are mapped to SBUF partitions in 16-row blocks (partition
p = row // 16), so every DMA descriptor is a large contiguous run
(8 KiB for f32 row-blocks, 1 KiB for the packed cache, 64 B for
scale/zero vectors) instead of the 512 B / 4 B runs a row-interleaved
layout produces.

The packed cache itself is never returned, so the [start, end) region is
quantize->dequantized entirely on-chip; only the [0, start) prefix is read
from the cache inputs.

Work is spread over three engines: DVE (min/max reduces, stats, nibble
unpack, part of the dequant), ACT (quantize with RNE via i32-convert,
part of the dequant), GPSIMD (mult+add dequant chunks).
"""

import sys

sys.path.insert(0, "/opt/trn_rl_repo")

import numpy as np
from concourse import bass, mybir
from concourse import tile
from concourse.bass_utils import run_bass_kernel_spmd

F32 = mybir.dt.float32
I32 = mybir.dt.int32
U32 = mybir.dt.uint32
U8 = mybir.dt.uint8
Alu = mybir.AluOpType
Act = mybir.ActivationFunctionType
AX = mybir.AxisListType
INV15 = float(np.float32(1.0 / 15.0))

B, H, L, D = 2, 32, 2048, 128
MAX_SEQ = 8192
N_CORES = 8
HC = H // N_CORES  # heads per core
RB = 16            # rows per partition block


def _split_multiwait(nc):
    """This container's walrus accepts only ONE sync-wait per instruction;
    Tile's tail drain (and occasionally other insts) carry several. Split
    extras into single-wait EventSemaphore insts inserted just before."""
    for fn in nc.m.functions:
        for blk in fn.blocks:
            out = []
            for ins in blk.instructions:
                si = ins.sync_info
                if si is not None and si.on_wait is not None and len(si.on_wait) > 1:
                    waits = list(si.on_wait)
                    for j, w in enumerate(waits[:-1]):
                        out.append(mybir.InstEventSemaphore(
                            name=f"{ins.name}_sw{j}", ins=[], outs=[],
                            engine=ins.engine,
                            sync_info=mybir.SyncInfo(on_wait=[w], on_update=[])))
                    si.on_wait = [waits[-1]]
                    ins.sync_info = si
                out.append(ins)
            blk.instructions = out


# Engine assignment tables per (pair, tensor) unit index u = pair*2 + kv
# (16 units). "V" = vector/DVE, "A" = scalar/ACT, "G" = gpsimd.
# V's fixed work (reduce/stats/unpack) ends early, so V takes the LATE
# units' dequant blocks to flatten the pipeline drain; A/G take early ones.
DEQF_ENG = ["V"] * 6 + ["A"] * 4 + ["G"] * 3 + ["A"] * 1 + ["V"] * 2
DEQP_ENG = ["G"] * 14 + ["V"] * 2
QUANT_ENG = ["A"] * 14 + ["V"] * 2


def _build(start_pos: int):
    """Trace the per-core Bass kernel for a given start_pos (block layout).

    Per core: xk/xv (B,HC,L,D) f32, prefix packed caches (B,HC,S,64) u8 and
    prefix scale/zero rows (B,HC,S) f32 -> ok/ov (B,HC,S+L,D) f32.
    """
    S = start_pos
    E = S + L
    assert S == 2048 and L == 2048 and E <= MAX_SEQ

    nc = bass.Bass(trn_type="TRN2")

    ins_q, ins_p, ins_sc, ins_zp, outs = {}, {}, {}, {}, {}
    for t in ("k", "v"):
        ins_q[t] = nc.dram_tensor(f"x{t}", [B, HC, L, D], F32, kind="ExternalInput")
        ins_p[t] = nc.dram_tensor(f"p{t}", [B, HC, S, D // 2], U8, kind="ExternalInput")
        ins_sc[t] = nc.dram_tensor(f"sc{t}", [B, HC, S], F32, kind="ExternalInput")
        ins_zp[t] = nc.dram_tensor(f"zp{t}", [B, HC, S], F32, kind="ExternalInput")
        outs[t] = nc.dram_tensor(f"o{t}", [B, HC, E, D], F32, kind="ExternalOutput")

    def chunk_op(eng, out, in0, s1, s2):
        """out = in0*s1 + s2 on the given engine (dequant chunk)."""
        if eng == "V":
            nc.vector.tensor_scalar(out=out, in0=in0, scalar1=s1, scalar2=s2,
                                    op0=Alu.mult, op1=Alu.add)
        elif eng == "G":
            nc.gpsimd.tensor_scalar(out=out, in0=in0, scalar1=s1, scalar2=s2,
                                    op0=Alu.mult, op1=Alu.add)
        else:
            nc.scalar.activation(out=out, in_=in0, func=Act.Identity,
                                 bias=s2, scale=s1)

    with tile.TileContext(nc) as tc:
        with tc.tile_pool(name="xin", bufs=3) as xin, \
             tc.tile_pool(name="qp", bufs=2) as qp, \
             tc.tile_pool(name="op", bufs=2) as op_pool, \
             tc.tile_pool(name="small", bufs=3) as small:
            NP = B * HC
            pairs = [(b, hh) for b in range(B) for hh in range(HC)]
            st = [dict() for _ in range(NP)]  # per-pair tile state

            def emit_dma_in(i):
                b, hh = pairs[i]
                p = st[i]
                for kv, t in enumerate(("k", "v")):
                    x_dram = ins_q[t][b, hh, :, :].rearrange("(p r) d -> p r d", p=128)
                    x = xin.tile([128, RB, D], F32, tag=f"x{kv}", name=f"x{kv}")
                    nc.sync.dma_start(out=x[:, :, :], in_=x_dram)
                    p[f"x{kv}"] = x
                    pk_dram = ins_p[t][b, hh, :, :].rearrange("(p r) d -> p r d", p=128)
                    pk = xin.tile([128, RB, D // 2], U8, tag=f"pk{kv}", name=f"pk{kv}")
                    nc.sync.dma_start(out=pk[:, :, :], in_=pk_dram)
                    p[f"pk{kv}"] = pk
                sc = small.tile([128, 2 * RB], F32, tag="sc", name="sc")
                zp = small.tile([128, 2 * RB], F32, tag="zp", name="zp")
                for kv, t in enumerate(("k", "v")):
                    c0 = kv * RB
                    nc.sync.dma_start(out=sc[:, c0:c0 + RB],
                                      in_=ins_sc[t][b, hh, :].rearrange("(p r) -> p r", p=128))
                    nc.sync.dma_start(out=zp[:, c0:c0 + RB],
                                      in_=ins_zp[t][b, hh, :].rearrange("(p r) -> p r", p=128))
                p["sc"], p["zp"] = sc, zp

            def emit_stage1(i):
                """V-engine work for pair i: reduces, stats, unpack, pnzs."""
                p = st[i]
                mn = small.tile([128, 2 * RB], F32, tag="mn", name="mn")
                mx = small.tile([128, 2 * RB], F32, tag="mx", name="mx")
                for kv in range(2):
                    x = p[f"x{kv}"]
                    c0 = kv * RB
                    nc.vector.tensor_reduce(out=mx[:, c0:c0 + RB], in_=x[:, :, :],
                                            axis=AX.X, op=Alu.max)
                    nc.vector.tensor_reduce(out=mn[:, c0:c0 + RB], in_=x[:, :, :],
                                            axis=AX.X, op=Alu.min)
                scale = small.tile([128, 2 * RB], F32, tag="scale", name="scale")
                nc.vector.tensor_tensor(out=scale[:, :], in0=mx[:, :], in1=mn[:, :],
                                        op=Alu.subtract)
                nc.vector.tensor_scalar(out=scale[:, :], in0=scale[:, :],
                                        scalar1=INV15, scalar2=1e-8,
                                        op0=Alu.mult, op1=Alu.max)
                rcp = small.tile([128, 2 * RB], F32, tag="rcp", name="rcp")
                nc.vector.reciprocal(out=rcp[:, :], in_=scale[:, :])
                zero = small.tile([128, 2 * RB], F32, tag="zero", name="zero")
                nc.vector.scalar_tensor_tensor(out=zero[:, :], in0=mn[:, :],
                                               scalar=-1.0, in1=rcp[:, :],
                                               op0=Alu.mult, op1=Alu.mult)
                nzs = small.tile([128, 2 * RB], F32, tag="nzs", name="nzs")
                nc.vector.scalar_tensor_tensor(out=nzs[:, :], in0=zero[:, :],
                                               scalar=-1.0, in1=scale[:, :],
                                               op0=Alu.mult, op1=Alu.mult)
                pnzs = small.tile([128, 2 * RB], F32, tag="pnzs", name="pnzs")
                nc.vector.scalar_tensor_tensor(out=pnzs[:, :], in0=p["zp"][:, :],
                                               scalar=-1.0, in1=p["sc"][:, :],
                                               op0=Alu.mult, op1=Alu.mult)
                p.update(scale=scale, rcp=rcp, zero=zero, nzs=nzs, pnzs=pnzs)
                for kv in range(2):
                    pk = p[f"pk{kv}"]
                    lohi = xin.tile([128, RB, D], U8, tag=f"lohi{kv}", name=f"lohi{kv}")
                    pk32 = pk[:, :, :].bitcast(U32)
                    nc.vector.tensor_scalar(out=lohi[:, :, 0:D // 2].bitcast(U32),
                                            in0=pk32, scalar1=0x0F0F0F0F,
                                            scalar2=None, op0=Alu.bitwise_and)
                    nc.vector.tensor_scalar(out=lohi[:, :, D // 2:D].bitcast(U32),
                                            in0=pk32, scalar1=4, scalar2=0x0F0F0F0F,
                                            op0=Alu.logical_shift_right,
                                            op1=Alu.bitwise_and)
                    p[f"lohi{kv}"] = lohi

            def emit_quant(i):
                """A-engine (or V) quantize for pair i."""
                p = st[i]
                for kv in range(2):
                    u = i * 2 + kv
                    x = p[f"x{kv}"]
                    c0 = kv * RB
                    q = qp.tile([128, RB, D], I32, tag=f"q{kv}", name=f"q{kv}")
                    for r in range(RB):
                        col = c0 + r
                        if QUANT_ENG[u] == "A":
                            nc.scalar.activation(out=q[:, r, :], in_=x[:, r, :],
                                                 func=Act.Identity,
                                                 bias=p["zero"][:, col:col + 1],
                                                 scale=p["rcp"][:, col:col + 1])
                        else:
                            nc.vector.tensor_scalar(out=q[:, r, :], in0=x[:, r, :],
                                                    scalar1=p["rcp"][:, col:col + 1],
                                                    scalar2=p["zero"][:, col:col + 1],
                                                    op0=Alu.mult, op1=Alu.add)
                    p[f"q{kv}"] = q

            def emit_deqP(i):
                """G/V prefix dequant for pair i + prefix out-DMA."""
                b, hh = pairs[i]
                p = st[i]
                for kv, t in enumerate(("k", "v")):
                    u = i * 2 + kv
                    c0 = kv * RB
                    o = op_pool.tile([128, 2 * RB, D], F32, tag=f"o{kv}", name=f"o{kv}")
                    p[f"o{kv}"] = o
                    lohi = p[f"lohi{kv}"]
                    for r in range(RB):
                        col = c0 + r
                        src = lohi[:, r, :].rearrange("p (two d) -> p two d", two=2)
                        dst = o[:, r, :].rearrange("p (d two) -> p two d", two=2)
                        chunk_op(DEQP_ENG[u], dst, src,
                                 p["sc"][:, col:col + 1], p["pnzs"][:, col:col + 1])
                    opre_dram = outs[t][b, hh, 0:S, :].rearrange("(p r) d -> p r d", p=128)
                    nc.sync.dma_start(out=opre_dram, in_=o[:, 0:RB, :])

            def emit_deqF(i, engines):
                """Fresh dequant blocks of pair i whose engine is in `engines`;
                emits the fresh-half out-DMA after the later block."""
                b, hh = pairs[i]
                p = st[i]
                for kv, t in enumerate(("k", "v")):
                    u = i * 2 + kv
                    if DEQF_ENG[u] not in engines:
                        continue
                    c0 = kv * RB
                    o = p[f"o{kv}"]
                    q = p[f"q{kv}"]
                    for r in range(RB):
                        col = c0 + r
                        chunk_op(DEQF_ENG[u], o[:, RB + r, :], q[:, r, :],
                                 p["scale"][:, col:col + 1], p["nzs"][:, col:col + 1])
                    ofr_dram = outs[t][b, hh, S:E, :].rearrange("(p r) d -> p r d", p=128)
                    nc.sync.dma_start(out=ofr_dram, in_=o[:, RB:2 * RB, :])

            emit_dma_in(0)
            emit_dma_in(1)
            for i in range(NP):
                if i >= 1:
                    emit_deqP(i - 1)
                    emit_deqF(i - 1, ("A",))
                emit_stage1(i)
                emit_quant(i)
                if i + 2 < NP:
                    emit_dma_in(i + 2)
                if i >= 1:
                    emit_deqF(i - 1, ("V", "G"))
            emit_deqP(NP - 1)
            emit_deqF(NP - 1, ("A",))
            emit_deqF(NP - 1, ("V", "G"))

    _split_multiwait(nc)
    return nc


_CACHE = {}


def _get_nc(start_pos: int):
    if start_pos not in _CACHE:
        _CACHE[start_pos] = _build(start_pos)
    return _CACHE[start_pos]


def _install_ntff_hook_shim():
    """The agent image's antenv lacks axon_hooks; recreate it so
    run_bass_kernel_spmd(trace=True) can drive NTFF profiling."""
    import types
    if "antenv.axon_hooks" in sys.modules:
        return
    mod = types.ModuleType("antenv.axon_hooks")
    state = {"hook": None}
    try:
        from trn_agent_boot.trn_boot import _ntff_profile_via_ctypes
        state["hook"] = _ntff_profile_via_ctypes("/opt/axon/libaxon_pjrt.so")
    except Exception:
        pass
    mod.get_axon_ntff_profile_hook = lambda: state["hook"]
    mod.set_axon_ntff_profile_hook = lambda h: state.__setitem__("hook", h)
    sys.modules["antenv.axon_hooks"] = mod


def _kernel_np(k, v, k_cache, v_cache, k_scale, k_zero, v_scale, v_zero, start_pos):
    """Pure-numpy fallback for shapes the bass path doesn't handle."""
    def qp(x):
        mn = x.min(-1, keepdims=True)
        mx = x.max(-1, keepdims=True)
        scale = np.maximum((mx - mn) / np.float32(15.0), np.float32(1e-8))
        zero = -mn / scale
        q = np.clip(np.round(x / scale + zero), 0, 15).astype(np.uint8)
        return (q[..., 0::2] | (q[..., 1::2] << 4)), scale[..., 0], zero[..., 0]

    def dq(p, s, z):
        lo = (p & 15).astype(np.float32)
        hi = ((p >> 4) & 15).astype(np.float32)
        q = np.stack([lo, hi], -1).reshape(p.shape[:-1] + (p.shape[-1] * 2,))
        return (q - z[..., None]) * s[..., None]

    S = int(start_pos)
    E = S + k.shape[2]
    outs = []
    for x, cache, sc, zp in ((k, k_cache, k_scale, k_zero), (v, v_cache, v_scale, v_zero)):
        pp, ps, pz = qp(x)
        cache = cache.copy(); sc = sc.copy(); zp = zp.copy()
        cache[:, :, S:E] = pp
        sc[:, :, S:E] = ps
        zp[:, :, S:E] = pz
        outs.append(dq(cache[:, :, :E], sc[:, :, :E], zp[:, :, :E]))
    return tuple(outs)


def kernel(k, v, k_cache, v_cache, k_scale, k_zero, v_scale, v_zero, start_pos,
           _trace=False):
    k = np.asarray(k, np.float32)
    v = np.asarray(v, np.float32)
    k_cache = np.asarray(k_cache, np.uint8)
    v_cache = np.asarray(v_cache, np.uint8)
    k_scale = np.asarray(k_scale, np.float32)
    k_zero = np.asarray(k_zero, np.float32)
    v_scale = np.asarray(v_scale, np.float32)
    v_zero = np.asarray(v_zero, np.float32)
    S = int(start_pos)

    if k.shape != (B, H, L, D) or S != 2048 or S + L > MAX_SEQ:
        return _kernel_np(k, v, k_cache, v_cache, k_scale, k_zero, v_scale, v_zero, S)

    nc = _get_nc(S)
    E = S + L

    in_maps = []
    for m in range(N_CORES):
        hs = slice(m * HC, (m + 1) * HC)
        im = {
            "xk": np.ascontiguousarray(k[:, hs]),
            "xv": np.ascontiguousarray(v[:, hs]),
            "pk": np.ascontiguousarray(k_cache[:, hs, :S, :]),
            "pv": np.ascontiguousarray(v_cache[:, hs, :S, :]),
            "sck": np.ascontiguousarray(k_scale[:, hs, :S]),
            "zpk": np.ascontiguousarray(k_zero[:, hs, :S]),
            "scv": np.ascontiguousarray(v_scale[:, hs, :S]),
            "zpv": np.ascontiguousarray(v_zero[:, hs, :S]),
        }
        in_maps.append(im)

    if _trace:
        _install_ntff_hook_shim()
    res = run_bass_kernel_spmd(nc, in_maps, list(range(N_CORES)), trace=_trace)

    k_dec = np.empty((B, H, E, D), np.float32)
    v_dec = np.empty((B, H, E, D), np.float32)
    for m in range(N_CORES):
        hs = slice(m * HC, (m + 1) * HC)
        k_dec[:, hs] = res.results[m]["ok"]
        v_dec[:, hs] = res.results[m]["ov"]
    if _trace:
        return (k_dec, v_dec), res
    return k_dec, v_dec


# revision 13
# speedup vs baseline: 1.1335x; 1.0023x over previous
"""CompressedKVCache kernel for Trainium2 (8 NeuronCores, head-sharded).

Computes, per (b, h) head:
  quantize k/v rows to int4 (per-row min/max affine), scatter into a
  uint8-packed cache at [start_pos : start_pos+L), then dequantize the
  cache prefix [0 : start_pos+L) back to f32.

Sharding: H=32 heads split across 8 cores (4 heads each); everything is
independent per head, no cross-core communication.

Layout: rows are mapped to SBUF partitions in 16-row blocks (partition
p = row // 16), so every DMA descriptor is a large contiguous run
(8 KiB for f32 row-blocks, 1 KiB for the packed cache, 64 B for
scale/zero vectors) instead of the 512 B / 4 B runs a row-interleaved
layout produces.

The packed cache itself is never returned, so the [start, end) region is
quantize->dequantized entirely on-chip; only the [0, start) prefix is read
from the cache inputs.

Work is spread over three engines: DVE (min/max reduces, stats, nibble
unpack, part of the dequant), ACT (quantize with RNE via i32-convert,
part of the dequant), GPSIMD (mult+add dequant chunks).
"""

import sys

sys.path.insert(0, "/opt/trn_rl_repo")

import numpy as np
from concourse import bass, mybir
from concourse import tile
from concourse.bass_utils import run_bass_kernel_spmd

F32 = mybir.dt.float32
I32 = mybir.dt.int32
U32 = mybir.dt.uint32
U8 = mybir.dt.uint8
Alu = mybir.AluOpType
Act = mybir.ActivationFunctionType
AX = mybir.AxisListType
INV15 = float(np.float32(1.0 / 15.0))

B, H, L, D = 2, 32, 2048, 128
MAX_SEQ = 8192
N_CORES = 8
HC = H // N_CORES  # heads per core
RB = 16            # rows per partition block


def _split_multiwait(nc):
    """This container's walrus accepts only ONE sync-wait per instruction;
    Tile's tail drain (and occasionally other insts) carry several. Split
    extras into single-wait EventSemaphore insts inserted just before."""
    for fn in nc.m.functions:
        for blk in fn.blocks:
            out = []
            for ins in blk.instructions:
                si = ins.sync_info
                if si is not None and si.on_wait is not None and len(si.on_wait) > 1:
                    waits = list(si.on_wait)
                    for j, w in enumerate(waits[:-1]):
                        out.append(mybir.InstEventSemaphore(
                            name=f"{ins.name}_sw{j}", ins=[], outs=[],
                            engine=ins.engine,
                            sync_info=mybir.SyncInfo(on_wait=[w], on_update=[])))
                    si.on_wait = [waits[-1]]
                    ins.sync_info = si
                out.append(ins)
            blk.instructions = out


# Engine assignment tables per (pair, tensor) unit index u = pair*2 + kv
# (16 units). "V" = vector/DVE, "A" = scalar/ACT, "G" = gpsimd.
# V's fixed work (reduce/stats/unpack) ends early, so V takes the LATE
# units' dequant blocks to flatten the pipeline drain; A/G take early ones.
DEQF_ENG = ["V"] * 6 + ["A"] * 4 + ["G"] * 3 + ["A"] * 1 + ["V"] * 2
DEQP_ENG = ["G"] * 14 + ["V"] * 2
QUANT_ENG = ["A"] * 14 + ["V"] * 2


def _build(start_pos: int):
    """Trace the per-core Bass kernel for a given start_pos (block layout).

    Per core: xk/xv (B,HC,L,D) f32, prefix packed caches (B,HC,S,64) u8 and
    prefix scale/zero rows (B,HC,S) f32 -> ok/ov (B,HC,S+L,D) f32.
    """
    S = start_pos
    E = S + L
    assert S == 2048 and L == 2048 and E <= MAX_SEQ

    nc = bass.Bass(trn_type="TRN2")

    ins_q, ins_p, ins_sc, ins_zp, outs = {}, {}, {}, {}, {}
    for t in ("k", "v"):
        ins_q[t] = nc.dram_tensor(f"x{t}", [B, HC, L, D], F32, kind="ExternalInput")
        ins_p[t] = nc.dram_tensor(f"p{t}", [B, HC, S, D // 2], U8, kind="ExternalInput")
        ins_sc[t] = nc.dram_tensor(f"sc{t}", [B, HC, S], F32, kind="ExternalInput")
        ins_zp[t] = nc.dram_tensor(f"zp{t}", [B, HC, S], F32, kind="ExternalInput")
        outs[t] = nc.dram_tensor(f"o{t}", [B, HC, E, D], F32, kind="ExternalOutput")

    def chunk_op(eng, out, in0, s1, s2):
        """out = in0*s1 + s2 on the given engine (dequant chunk)."""
        if eng == "V":
            nc.vector.tensor_scalar(out=out, in0=in0, scalar1=s1, scalar2=s2,
                                    op0=Alu.mult, op1=Alu.add)
        elif eng == "G":
            nc.gpsimd.tensor_scalar(out=out, in0=in0, scalar1=s1, scalar2=s2,
                                    op0=Alu.mult, op1=Alu.add)
        else:
            nc.scalar.activation(out=out, in_=in0, func=Act.Identity,
                                 bias=s2, scale=s1)

    with tile.TileContext(nc) as tc:
        with tc.tile_pool(name="xin", bufs=3) as xin, \
             tc.tile_pool(name="qp", bufs=2) as qp, \
             tc.tile_pool(name="op", bufs=2) as op_pool, \
             tc.tile_pool(name="small", bufs=3) as small:
            NP = B * HC
            pairs = [(b, hh) for b in range(B) for hh in range(HC)]
            st = [dict() for _ in range(NP)]  # per-pair tile state

            def emit_dma_in(i):
                b, hh = pairs[i]
                p = st[i]
                for kv, t in enumerate(("k", "v")):
                    x_dram = ins_q[t][b, hh, :, :].rearrange("(p r) d -> p r d", p=128)
                    x = xin.tile([128, RB, D], F32, tag=f"x{kv}", name=f"x{kv}")
                    nc.sync.dma_start(out=x[:, :, :], in_=x_dram)
                    p[f"x{kv}"] = x
                    pk_dram = ins_p[t][b, hh, :, :].rearrange("(p r) d -> p r d", p=128)
                    pk = xin.tile([128, RB, D // 2], U8, tag=f"pk{kv}", name=f"pk{kv}")
                    nc.sync.dma_start(out=pk[:, :, :], in_=pk_dram)
                    p[f"pk{kv}"] = pk
                sc = small.tile([128, 2 * RB], F32, tag="sc", name="sc")
                zp = small.tile([128, 2 * RB], F32, tag="zp", name="zp")
                for kv, t in enumerate(("k", "v")):
                    c0 = kv * RB
                    nc.sync.dma_start(out=sc[:, c0:c0 + RB],
                                      in_=ins_sc[t][b, hh, :].rearrange("(p r) -> p r", p=128))
                    nc.sync.dma_start(out=zp[:, c0:c0 + RB],
                                      in_=ins_zp[t][b, hh, :].rearrange("(p r) -> p r", p=128))
                p["sc"], p["zp"] = sc, zp

            def emit_stage1(i):
                """V-engine work for pair i: reduces, stats, unpack, pnzs."""
                p = st[i]
                mn = small.tile([128, 2 * RB], F32, tag="mn", name="mn")
                mx = small.tile([128, 2 * RB], F32, tag="mx", name="mx")
                for kv in range(2):
                    x = p[f"x{kv}"]
                    c0 = kv * RB
                    nc.vector.tensor_reduce(out=mx[:, c0:c0 + RB], in_=x[:, :, :],
                                            axis=AX.X, op=Alu.max)
                    nc.vector.tensor_reduce(out=mn[:, c0:c0 + RB], in_=x[:, :, :],
                                            axis=AX.X, op=Alu.min)
                scale = small.tile([128, 2 * RB], F32, tag="scale", name="scale")
                nc.vector.tensor_tensor(out=scale[:, :], in0=mx[:, :], in1=mn[:, :],
                                        op=Alu.subtract)
                nc.vector.tensor_scalar(out=scale[:, :], in0=scale[:, :],
                                        scalar1=INV15, scalar2=1e-8,
                                        op0=Alu.mult, op1=Alu.max)
                rcp = small.tile([128, 2 * RB], F32, tag="rcp", name="rcp")
                nc.vector.reciprocal(out=rcp[:, :], in_=scale[:, :])
                zero = small.tile([128, 2 * RB], F32, tag="zero", name="zero")
                nc.vector.scalar_tensor_tensor(out=zero[:, :], in0=mn[:, :],
                                               scalar=-1.0, in1=rcp[:, :],
                                               op0=Alu.mult, op1=Alu.mult)
                nzs = small.tile([128, 2 * RB], F32, tag="nzs", name="nzs")
                nc.vector.scalar_tensor_tensor(out=nzs[:, :], in0=zero[:, :],
                                               scalar=-1.0, in1=scale[:, :],
                                               op0=Alu.mult, op1=Alu.mult)
                pnzs = small.tile([128, 2 * RB], F32, tag="pnzs", name="pnzs")
                nc.vector.scalar_tensor_tensor(out=pnzs[:, :], in0=p["zp"][:, :],
                                               scalar=-1.0, in1=p["sc"][:, :],
                                               op0=Alu.mult, op1=Alu.mult)
                p.update(scale=scale, rcp=rcp, zero=zero, nzs=nzs, pnzs=pnzs)
                for kv in range(2):
                    pk = p[f"pk{kv}"]
                    lohi = xin.tile([128, RB, D], U8, tag=f"lohi{kv}", name=f"lohi{kv}")
                    pk32 = pk[:, :, :].bitcast(U32)
                    nc.vector.tensor_scalar(out=lohi[:, :, 0:D // 2].bitcast(U32),
                                            in0=pk32, scalar1=0x0F0F0F0F,
                                            scalar2=None, op0=Alu.bitwise_and)
                    nc.vector.tensor_scalar(out=lohi[:, :, D // 2:D].bitcast(U32),
                                            in0=pk32, scalar1=4, scalar2=0x0F0F0F0F,
                                            op0=Alu.logical_shift_right,
                                            op1=Alu.bitwise_and)
                    p[f"lohi{kv}"] = lohi

            def emit_quant(i):
                """A-engine (or V) quantize for pair i."""
                p = st[i]
                for kv in range(2):
                    u = i * 2 + kv
                    x = p[f"x{kv}"]
                    c0 = kv * RB
                    q = qp.tile([128, RB, D], I32, tag=f"q{kv}", name=f"q{kv}")
                    for r in range(RB):
                        col = c0 + r
                        if QUANT_ENG[u] == "A":
                            nc.scalar.activation(out=q[:, r, :], in_=x[:, r, :],
                                                 func=Act.Identity,
                                                 bias=p["zero"][:, col:col + 1],
                                                 scale=p["rcp"][:, col:col + 1])
                        else:
                            nc.vector.tensor_scalar(out=q[:, r, :], in0=x[:, r, :],
                                                    scalar1=p["rcp"][:, col:col + 1],
                                                    scalar2=p["zero"][:, col:col + 1],
                                                    op0=Alu.mult, op1=Alu.add)
                    p[f"q{kv}"] = q

            def emit_deqP(i):
                """G/V prefix dequant for pair i + prefix out-DMA."""
                b, hh = pairs[i]
                p = st[i]
                for kv, t in enumerate(("k", "v")):
                    u = i * 2 + kv
                    c0 = kv * RB
                    o = op_pool.tile([128, 2 * RB, D], F32, tag=f"o{kv}", name=f"o{kv}")
                    p[f"o{kv}"] = o
                    lohi = p[f"lohi{kv}"]
                    for r in range(RB):
                        col = c0 + r
                        src = lohi[:, r, :].rearrange("p (two d) -> p two d", two=2)
                        dst = o[:, r, :].rearrange("p (d two) -> p two d", two=2)
                        chunk_op(DEQP_ENG[u], dst, src,
                                 p["sc"][:, col:col + 1], p["pnzs"][:, col:col + 1])

            def emit_deqF(i, engines):
                """Fresh dequant blocks of pair i whose engine is in `engines`;
                emits the fresh-half out-DMA after the later block."""
                b, hh = pairs[i]
                p = st[i]
                for kv, t in enumerate(("k", "v")):
                    u = i * 2 + kv
                    if DEQF_ENG[u] not in engines:
                        continue
                    c0 = kv * RB
                    o = p[f"o{kv}"]
                    q = p[f"q{kv}"]
                    for r in range(RB):
                        col = c0 + r
                        chunk_op(DEQF_ENG[u], o[:, RB + r, :], q[:, r, :],
                                 p["scale"][:, col:col + 1], p["nzs"][:, col:col + 1])

            def emit_outs(i):
                b, hh = pairs[i]
                p = st[i]
                for kv, t in enumerate(("k", "v")):
                    o = p[f"o{kv}"]
                    opre_dram = outs[t][b, hh, 0:S, :].rearrange("(p r) d -> p r d", p=128)
                    nc.sync.dma_start(out=opre_dram, in_=o[:, 0:RB, :])
                    ofr_dram = outs[t][b, hh, S:E, :].rearrange("(p r) d -> p r d", p=128)
                    nc.sync.dma_start(out=ofr_dram, in_=o[:, RB:2 * RB, :])

            emit_dma_in(0)
            emit_dma_in(1)
            for i in range(NP):
                if i >= 1:
                    emit_deqP(i - 1)
                    emit_deqF(i - 1, ("A",))
                emit_stage1(i)
                emit_quant(i)
                if i + 2 < NP:
                    emit_dma_in(i + 2)
                if i >= 1:
                    emit_deqF(i - 1, ("V", "G"))
                    emit_outs(i - 1)
            emit_deqP(NP - 1)
            emit_deqF(NP - 1, ("A",))
            emit_deqF(NP - 1, ("V", "G"))
            emit_outs(NP - 1)

    _split_multiwait(nc)
    return nc


_CACHE = {}


def _get_nc(start_pos: int):
    if start_pos not in _CACHE:
        _CACHE[start_pos] = _build(start_pos)
    return _CACHE[start_pos]


def _install_ntff_hook_shim():
    """The agent image's antenv lacks axon_hooks; recreate it so
    run_bass_kernel_spmd(trace=True) can drive NTFF profiling."""
    import types
    if "antenv.axon_hooks" in sys.modules:
        return
    mod = types.ModuleType("antenv.axon_hooks")
    state = {"hook": None}
    try:
        from trn_agent_boot.trn_boot import _ntff_profile_via_ctypes
        state["hook"] = _ntff_profile_via_ctypes("/opt/axon/libaxon_pjrt.so")
    except Exception:
        pass
    mod.get_axon_ntff_profile_hook = lambda: state["hook"]
    mod.set_axon_ntff_profile_hook = lambda h: state.__setitem__("hook", h)
    sys.modules["antenv.axon_hooks"] = mod


def _kernel_np(k, v, k_cache, v_cache, k_scale, k_zero, v_scale, v_zero, start_pos):
    """Pure-numpy fallback for shapes the bass path doesn't handle."""
    def qp(x):
        mn = x.min(-1, keepdims=True)
        mx = x.max(-1, keepdims=True)
        scale = np.maximum((mx - mn) / np.float32(15.0), np.float32(1e-8))
        zero = -mn / scale
        q = np.clip(np.round(x / scale + zero), 0, 15).astype(np.uint8)
        return (q[..., 0::2] | (q[..., 1::2] << 4)), scale[..., 0], zero[..., 0]

    def dq(p, s, z):
        lo = (p & 15).astype(np.float32)
        hi = ((p >> 4) & 15).astype(np.float32)
        q = np.stack([lo, hi], -1).reshape(p.shape[:-1] + (p.shape[-1] * 2,))
        return (q - z[..., None]) * s[..., None]

    S = int(start_pos)
    E = S + k.shape[2]
    outs = []
    for x, cache, sc, zp in ((k, k_cache, k_scale, k_zero), (v, v_cache, v_scale, v_zero)):
        pp, ps, pz = qp(x)
        cache = cache.copy(); sc = sc.copy(); zp = zp.copy()
        cache[:, :, S:E] = pp
        sc[:, :, S:E] = ps
        zp[:, :, S:E] = pz
        outs.append(dq(cache[:, :, :E], sc[:, :, :E], zp[:, :, :E]))
    return tuple(outs)


def kernel(k, v, k_cache, v_cache, k_scale, k_zero, v_scale, v_zero, start_pos,
           _trace=False):
    k = np.asarray(k, np.float32)
    v = np.asarray(v, np.float32)
    k_cache = np.asarray(k_cache, np.uint8)
    v_cache = np.asarray(v_cache, np.uint8)
    k_scale = np.asarray(k_scale, np.float32)
    k_zero = np.asarray(k_zero, np.float32)
    v_scale = np.asarray(v_scale, np.float32)
    v_zero = np.asarray(v_zero, np.float32)
    S = int(start_pos)

    if k.shape != (B, H, L, D) or S != 2048 or S + L > MAX_SEQ:
        return _kernel_np(k, v, k_cache, v_cache, k_scale, k_zero, v_scale, v_zero, S)

    nc = _get_nc(S)
    E = S + L

    in_maps = []
    for m in range(N_CORES):
        hs = slice(m * HC, (m + 1) * HC)
        im = {
            "xk": np.ascontiguousarray(k[:, hs]),
            "xv": np.ascontiguousarray(v[:, hs]),
            "pk": np.ascontiguousarray(k_cache[:, hs, :S, :]),
            "pv": np.ascontiguousarray(v_cache[:, hs, :S, :]),
            "sck": np.ascontiguousarray(k_scale[:, hs, :S]),
            "zpk": np.ascontiguousarray(k_zero[:, hs, :S]),
            "scv": np.ascontiguousarray(v_scale[:, hs, :S]),
            "zpv": np.ascontiguousarray(v_zero[:, hs, :S]),
        }
        in_maps.append(im)

    if _trace:
        _install_ntff_hook_shim()
    res = run_bass_kernel_spmd(nc, in_maps, list(range(N_CORES)), trace=_trace)

    k_dec = np.empty((B, H, E, D), np.float32)
    v_dec = np.empty((B, H, E, D), np.float32)
    for m in range(N_CORES):
        hs = slice(m * HC, (m + 1) * HC)
        k_dec[:, hs] = res.results[m]["ok"]
        v_dec[:, hs] = res.results[m]["ov"]
    if _trace:
        return (k_dec, v_dec), res
    return k_dec, v_dec


# revision 14
# speedup vs baseline: 1.1585x; 1.0220x over previous
"""CompressedKVCache kernel for Trainium2 (8 NeuronCores, head-sharded).

Computes, per (b, h) head:
  quantize k/v rows to int4 (per-row min/max affine), scatter into a
  uint8-packed cache at [start_pos : start_pos+L), then dequantize the
  cache prefix [0 : start_pos+L) back to f32.

Sharding: H=32 heads split across 8 cores (4 heads each); everything is
independent per head, no cross-core communication.

Layout: rows are mapped to SBUF partitions in 16-row blocks (partition
p = row // 16), so every DMA descriptor is a large contiguous run
(8 KiB for f32 row-blocks, 1 KiB for the packed cache, 64 B for
scale/zero vectors) instead of the 512 B / 4 B runs a row-interleaved
layout produces.

The packed cache itself is never returned, so the [start, end) region is
quantize->dequantized entirely on-chip; only the [0, start) prefix is read
from the cache inputs.

Work is spread over three engines: DVE (min/max reduces, stats, nibble
unpack, part of the dequant), ACT (quantize with RNE via i32-convert,
part of the dequant), GPSIMD (mult+add dequant chunks).
"""

import sys

sys.path.insert(0, "/opt/trn_rl_repo")

import numpy as np
from concourse import bass, mybir
from concourse import tile
from concourse.bass_utils import run_bass_kernel_spmd

F32 = mybir.dt.float32
I32 = mybir.dt.int32
U32 = mybir.dt.uint32
U8 = mybir.dt.uint8
Alu = mybir.AluOpType
Act = mybir.ActivationFunctionType
AX = mybir.AxisListType
INV15 = float(np.float32(1.0 / 15.0))

B, H, L, D = 2, 32, 2048, 128
MAX_SEQ = 8192
N_CORES = 8
HC = H // N_CORES  # heads per core
RB = 16            # rows per partition block


def _split_multiwait(nc):
    """This container's walrus accepts only ONE sync-wait per instruction;
    Tile's tail drain (and occasionally other insts) carry several. Split
    extras into single-wait EventSemaphore insts inserted just before."""
    for fn in nc.m.functions:
        for blk in fn.blocks:
            out = []
            for ins in blk.instructions:
                si = ins.sync_info
                if si is not None and si.on_wait is not None and len(si.on_wait) > 1:
                    waits = list(si.on_wait)
                    for j, w in enumerate(waits[:-1]):
                        out.append(mybir.InstEventSemaphore(
                            name=f"{ins.name}_sw{j}", ins=[], outs=[],
                            engine=ins.engine,
                            sync_info=mybir.SyncInfo(on_wait=[w], on_update=[])))
                    si.on_wait = [waits[-1]]
                    ins.sync_info = si
                out.append(ins)
            blk.instructions = out


# Engine assignment tables per (pair, tensor) unit index u = pair*2 + kv
# (16 units). "V" = vector/DVE, "A" = scalar/ACT, "G" = gpsimd.
# V's fixed work (reduce/stats/unpack) ends early, so V takes the LATE
# units' dequant blocks to flatten the pipeline drain; A/G take early ones.
DEQF_ENG = ["V"] * 10 + ["G"] * 3 + ["A"] * 3
DEQP_ENG = ["G"] * 16
QUANT_ENG = ["A"] * 16


def _build(start_pos: int):
    """Trace the per-core Bass kernel for a given start_pos (block layout).

    Per core: xk/xv (B,HC,L,D) f32, prefix packed caches (B,HC,S,64) u8 and
    prefix scale/zero rows (B,HC,S) f32 -> ok/ov (B,HC,S+L,D) f32.
    """
    S = start_pos
    E = S + L
    assert S == 2048 and L == 2048 and E <= MAX_SEQ

    nc = bass.Bass(trn_type="TRN2")

    ins_q, ins_p, ins_sc, ins_zp, outs = {}, {}, {}, {}, {}
    for t in ("k", "v"):
        ins_q[t] = nc.dram_tensor(f"x{t}", [B, HC, L, D], F32, kind="ExternalInput")
        ins_p[t] = nc.dram_tensor(f"p{t}", [B, HC, S, D // 2], U8, kind="ExternalInput")
        ins_sc[t] = nc.dram_tensor(f"sc{t}", [B, HC, S], F32, kind="ExternalInput")
        ins_zp[t] = nc.dram_tensor(f"zp{t}", [B, HC, S], F32, kind="ExternalInput")
        outs[t] = nc.dram_tensor(f"o{t}", [B, HC, E, D], F32, kind="ExternalOutput")

    def chunk_op(eng, out, in0, s1, s2):
        """out = in0*s1 + s2 on the given engine (dequant chunk)."""
        if eng == "V":
            nc.vector.tensor_scalar(out=out, in0=in0, scalar1=s1, scalar2=s2,
                                    op0=Alu.mult, op1=Alu.add)
        elif eng == "G":
            nc.gpsimd.tensor_scalar(out=out, in0=in0, scalar1=s1, scalar2=s2,
                                    op0=Alu.mult, op1=Alu.add)
        else:
            nc.scalar.activation(out=out, in_=in0, func=Act.Identity,
                                 bias=s2, scale=s1)

    with tile.TileContext(nc) as tc:
        with tc.tile_pool(name="xin", bufs=3) as xin, \
             tc.tile_pool(name="qp", bufs=2) as qp, \
             tc.tile_pool(name="op", bufs=2) as op_pool, \
             tc.tile_pool(name="small", bufs=3) as small:
            NP = B * HC
            pairs = [(b, hh) for b in range(B) for hh in range(HC)]
            st = [dict() for _ in range(NP)]  # per-pair tile state

            def emit_dma_in(i):
                b, hh = pairs[i]
                p = st[i]
                for kv, t in enumerate(("k", "v")):
                    x_dram = ins_q[t][b, hh, :, :].rearrange("(p r) d -> p r d", p=128)
                    x = xin.tile([128, RB, D], F32, tag=f"x{kv}", name=f"x{kv}")
                    nc.sync.dma_start(out=x[:, :, :], in_=x_dram)
                    p[f"x{kv}"] = x
                    pk_dram = ins_p[t][b, hh, :, :].rearrange("(p r) d -> p r d", p=128)
                    pk = xin.tile([128, RB, D // 2], U8, tag=f"pk{kv}", name=f"pk{kv}")
                    nc.sync.dma_start(out=pk[:, :, :], in_=pk_dram)
                    p[f"pk{kv}"] = pk
                sc = small.tile([128, 2 * RB], F32, tag="sc", name="sc")
                zp = small.tile([128, 2 * RB], F32, tag="zp", name="zp")
                for kv, t in enumerate(("k", "v")):
                    c0 = kv * RB
                    nc.sync.dma_start(out=sc[:, c0:c0 + RB],
                                      in_=ins_sc[t][b, hh, :].rearrange("(p r) -> p r", p=128))
                    nc.sync.dma_start(out=zp[:, c0:c0 + RB],
                                      in_=ins_zp[t][b, hh, :].rearrange("(p r) -> p r", p=128))
                p["sc"], p["zp"] = sc, zp

            def emit_stage1(i):
                """V-engine work for pair i: reduces, stats, unpack, pnzs."""
                p = st[i]
                mn = small.tile([128, 2 * RB], F32, tag="mn", name="mn")
                mx = small.tile([128, 2 * RB], F32, tag="mx", name="mx")
                for kv in range(2):
                    x = p[f"x{kv}"]
                    c0 = kv * RB
                    nc.vector.tensor_reduce(out=mx[:, c0:c0 + RB], in_=x[:, :, :],
                                            axis=AX.X, op=Alu.max)
                    nc.vector.tensor_reduce(out=mn[:, c0:c0 + RB], in_=x[:, :, :],
                                            axis=AX.X, op=Alu.min)
                scale = small.tile([128, 2 * RB], F32, tag="scale", name="scale")
                nc.vector.tensor_tensor(out=scale[:, :], in0=mx[:, :], in1=mn[:, :],
                                        op=Alu.subtract)
                nc.vector.tensor_scalar(out=scale[:, :], in0=scale[:, :],
                                        scalar1=INV15, scalar2=1e-8,
                                        op0=Alu.mult, op1=Alu.max)
                rcp = small.tile([128, 2 * RB], F32, tag="rcp", name="rcp")
                nc.vector.reciprocal(out=rcp[:, :], in_=scale[:, :])
                zero = small.tile([128, 2 * RB], F32, tag="zero", name="zero")
                nc.vector.scalar_tensor_tensor(out=zero[:, :], in0=mn[:, :],
                                               scalar=-1.0, in1=rcp[:, :],
                                               op0=Alu.mult, op1=Alu.mult)
                nzs = small.tile([128, 2 * RB], F32, tag="nzs", name="nzs")
                nc.vector.scalar_tensor_tensor(out=nzs[:, :], in0=zero[:, :],
                                               scalar=-1.0, in1=scale[:, :],
                                               op0=Alu.mult, op1=Alu.mult)
                pnzs = small.tile([128, 2 * RB], F32, tag="pnzs", name="pnzs")
                nc.vector.scalar_tensor_tensor(out=pnzs[:, :], in0=p["zp"][:, :],
                                               scalar=-1.0, in1=p["sc"][:, :],
                                               op0=Alu.mult, op1=Alu.mult)
                p.update(scale=scale, rcp=rcp, zero=zero, nzs=nzs, pnzs=pnzs)
                for kv in range(2):
                    pk = p[f"pk{kv}"]
                    lohi = xin.tile([128, RB, D], U8, tag=f"lohi{kv}", name=f"lohi{kv}")
                    pk32 = pk[:, :, :].bitcast(U32)
                    nc.vector.tensor_scalar(out=lohi[:, :, 0:D // 2].bitcast(U32),
                                            in0=pk32, scalar1=0x0F0F0F0F,
                                            scalar2=None, op0=Alu.bitwise_and)
                    nc.vector.tensor_scalar(out=lohi[:, :, D // 2:D].bitcast(U32),
                                            in0=pk32, scalar1=4, scalar2=0x0F0F0F0F,
                                            op0=Alu.logical_shift_right,
                                            op1=Alu.bitwise_and)
                    p[f"lohi{kv}"] = lohi

            def emit_quant(i):
                """A-engine (or V) quantize for pair i."""
                p = st[i]
                for kv in range(2):
                    u = i * 2 + kv
                    x = p[f"x{kv}"]
                    c0 = kv * RB
                    q = qp.tile([128, RB, D], I32, tag=f"q{kv}", name=f"q{kv}")
                    for r in range(RB):
                        col = c0 + r
                        if QUANT_ENG[u] == "A":
                            nc.scalar.activation(out=q[:, r, :], in_=x[:, r, :],
                                                 func=Act.Identity,
                                                 bias=p["zero"][:, col:col + 1],
                                                 scale=p["rcp"][:, col:col + 1])
                        else:
                            nc.vector.tensor_scalar(out=q[:, r, :], in0=x[:, r, :],
                                                    scalar1=p["rcp"][:, col:col + 1],
                                                    scalar2=p["zero"][:, col:col + 1],
                                                    op0=Alu.mult, op1=Alu.add)
                    p[f"q{kv}"] = q

            def emit_deqP(i):
                """G/V prefix dequant for pair i + prefix out-DMA."""
                b, hh = pairs[i]
                p = st[i]
                for kv, t in enumerate(("k", "v")):
                    u = i * 2 + kv
                    c0 = kv * RB
                    o = op_pool.tile([128, 2 * RB, D], F32, tag=f"o{kv}", name=f"o{kv}")
                    p[f"o{kv}"] = o
                    lohi = p[f"lohi{kv}"]
                    for r in range(RB):
                        col = c0 + r
                        src = lohi[:, r, :].rearrange("p (two d) -> p two d", two=2)
                        dst = o[:, r, :].rearrange("p (d two) -> p two d", two=2)
                        chunk_op(DEQP_ENG[u], dst, src,
                                 p["sc"][:, col:col + 1], p["pnzs"][:, col:col + 1])

            def emit_deqF(i, engines):
                """Fresh dequant blocks of pair i whose engine is in `engines`;
                emits the fresh-half out-DMA after the later block."""
                b, hh = pairs[i]
                p = st[i]
                for kv, t in enumerate(("k", "v")):
                    u = i * 2 + kv
                    if DEQF_ENG[u] not in engines:
                        continue
                    c0 = kv * RB
                    o = p[f"o{kv}"]
                    q = p[f"q{kv}"]
                    for r in range(RB):
                        col = c0 + r
                        chunk_op(DEQF_ENG[u], o[:, RB + r, :], q[:, r, :],
                                 p["scale"][:, col:col + 1], p["nzs"][:, col:col + 1])

            def emit_outs(i):
                b, hh = pairs[i]
                p = st[i]
                for kv, t in enumerate(("k", "v")):
                    o = p[f"o{kv}"]
                    opre_dram = outs[t][b, hh, 0:S, :].rearrange("(p r) d -> p r d", p=128)
                    nc.sync.dma_start(out=opre_dram, in_=o[:, 0:RB, :])
                    ofr_dram = outs[t][b, hh, S:E, :].rearrange("(p r) d -> p r d", p=128)
                    nc.sync.dma_start(out=ofr_dram, in_=o[:, RB:2 * RB, :])

            emit_dma_in(0)
            emit_dma_in(1)
            for i in range(NP):
                if i >= 1:
                    emit_deqP(i - 1)
                    emit_deqF(i - 1, ("A",))
                emit_stage1(i)
                emit_quant(i)
                if i + 2 < NP:
                    emit_dma_in(i + 2)
                if i >= 1:
                    emit_deqF(i - 1, ("V", "G"))
                    emit_outs(i - 1)
            emit_deqP(NP - 1)
            emit_deqF(NP - 1, ("A",))
            emit_deqF(NP - 1, ("V", "G"))
            emit_outs(NP - 1)

    _split_multiwait(nc)
    return nc


_CACHE = {}


def _get_nc(start_pos: int):
    if start_pos not in _CACHE:
        _CACHE[start_pos] = _build(start_pos)
    return _CACHE[start_pos]


def _install_ntff_hook_shim():
    """The agent image's antenv lacks axon_hooks; recreate it so
    run_bass_kernel_spmd(trace=True) can drive NTFF profiling."""
    import types
    if "antenv.axon_hooks" in sys.modules:
        return
    mod = types.ModuleType("antenv.axon_hooks")
    state = {"hook": None}
    try:
        from trn_agent_boot.trn_boot import _ntff_profile_via_ctypes
        state["hook"] = _ntff_profile_via_ctypes("/opt/axon/libaxon_pjrt.so")
    except Exception:
        pass
    mod.get_axon_ntff_profile_hook = lambda: state["hook"]
    mod.set_axon_ntff_profile_hook = lambda h: state.__setitem__("hook", h)
    sys.modules["antenv.axon_hooks"] = mod


def _kernel_np(k, v, k_cache, v_cache, k_scale, k_zero, v_scale, v_zero, start_pos):
    """Pure-numpy fallback for shapes the bass path doesn't handle."""
    def qp(x):
        mn = x.min(-1, keepdims=True)
        mx = x.max(-1, keepdims=True)
        scale = np.maximum((mx - mn) / np.float32(15.0), np.float32(1e-8))
        zero = -mn / scale
        q = np.clip(np.round(x / scale + zero), 0, 15).astype(np.uint8)
        return (q[..., 0::2] | (q[..., 1::2] << 4)), scale[..., 0], zero[..., 0]

    def dq(p, s, z):
        lo = (p & 15).astype(np.float32)
        hi = ((p >> 4) & 15).astype(np.float32)
        q = np.stack([lo, hi], -1).reshape(p.shape[:-1] + (p.shape[-1] * 2,))
        return (q - z[..., None]) * s[..., None]

    S = int(start_pos)
    E = S + k.shape[2]
    outs = []
    for x, cache, sc, zp in ((k, k_cache, k_scale, k_zero), (v, v_cache, v_scale, v_zero)):
        pp, ps, pz = qp(x)
        cache = cache.copy(); sc = sc.copy(); zp = zp.copy()
        cache[:, :, S:E] = pp
        sc[:, :, S:E] = ps
        zp[:, :, S:E] = pz
        outs.append(dq(cache[:, :, :E], sc[:, :, :E], zp[:, :, :E]))
    return tuple(outs)


def kernel(k, v, k_cache, v_cache, k_scale, k_zero, v_scale, v_zero, start_pos,
           _trace=False):
    k = np.asarray(k, np.float32)
    v = np.asarray(v, np.float32)
    k_cache = np.asarray(k_cache, np.uint8)
    v_cache = np.asarray(v_cache, np.uint8)
    k_scale = np.asarray(k_scale, np.float32)
    k_zero = np.asarray(k_zero, np.float32)
    v_scale = np.asarray(v_scale, np.float32)
    v_zero = np.asarray(v_zero, np.float32)
    S = int(start_pos)

    if k.shape != (B, H, L, D) or S != 2048 or S + L > MAX_SEQ:
        return _kernel_np(k, v, k_cache, v_cache, k_scale, k_zero, v_scale, v_zero, S)

    nc = _get_nc(S)
    E = S + L

    in_maps = []
    for m in range(N_CORES):
        hs = slice(m * HC, (m + 1) * HC)
        im = {
            "xk": np.ascontiguousarray(k[:, hs]),
            "xv": np.ascontiguousarray(v[:, hs]),
            "pk": np.ascontiguousarray(k_cache[:, hs, :S, :]),
            "pv": np.ascontiguousarray(v_cache[:, hs, :S, :]),
            "sck": np.ascontiguousarray(k_scale[:, hs, :S]),
            "zpk": np.ascontiguousarray(k_zero[:, hs, :S]),
            "scv": np.ascontiguousarray(v_scale[:, hs, :S]),
            "zpv": np.ascontiguousarray(v_zero[:, hs, :S]),
        }
        in_maps.append(im)

    if _trace:
        _install_ntff_hook_shim()
    res = run_bass_kernel_spmd(nc, in_maps, list(range(N_CORES)), trace=_trace)

    k_dec = np.empty((B, H, E, D), np.float32)
    v_dec = np.empty((B, H, E, D), np.float32)
    for m in range(N_CORES):
        hs = slice(m * HC, (m + 1) * HC)
        k_dec[:, hs] = res.results[m]["ok"]
        v_dec[:, hs] = res.results[m]["ov"]
    if _trace:
        return (k_dec, v_dec), res
    return k_dec, v_dec
